# revision 13
# baseline (speedup 1.0000x reference)
"""Trainium2 Bass kernel for nn_CanonicalCov1D (strided dual-projection covariance).

Math (reference):
  shift = W = 128, STRIDE = 8, L = T - 128 = 8064, NWIN = 993
  win1[b,n,:] = X[b, 8n : 8n+128],  win2[b,n,:] = X[b, 128+8n : 256+8n]
  proj_i = win_i @ weight_i  (per (LAT, C))
  cov[b,n,c] = mean_l[(proj1 - mean_l proj1) * (proj2 - mean_l proj2)] + bias

Key simplifications:
  * Centering projections over LAT == projecting with LAT-centered weights:
    center (and 1/LAT-scale) weights on the host, skip mean subtraction.
  * win2[n] == win_full[n+16] (shift = 16*STRIDE): one im2col serves both.
  * l-major weight layout ([w, l*64+c]) puts (l, c) on PSUM partitions, so
    the LAT-reduction is a constant-selector matmul accumulating in PSUM.
  * TWIN COLUMN-TILED SELECTORS: the selector output is only M=64, so the
    two chunks of a jp-pair run as two CONCURRENT col-tiled matmuls
    (tile_position (0,0) / (0,64)) into the two partition-halves of ONE
    PSUM bank.  No pre-add pass anywhere (GPSIMD fully idle); the lo+hi
    halves are summed on the host during unsharding (along with bias add
    and the [b,c,n] -> [b,n,c] permute).
  * PSUM rings: p1 chunks rotate through a 3-slot (3-bank) ring, p2 chunks
    through a 4-slot (4-bank) ring (pairs always adjacent); the twin
    selector accumulator takes the 8th bank.  ACT exits / DVE muls operate
    on 1024-col access patterns for low per-op overhead.

Per-core device pipeline (data-parallel over batch, 4 batches/core):
  1. dma_start_transpose builds winT [128(w), 1040(n)] bf16 from the
     overlapping-window view of X (xbar transpose).
  2. per (batch, 512-window tile t, jp of 8):
       p1 pair -> p1ring slots   (2 matmuls, bf16)
       p2 pair -> p2ring slots   (2 matmuls, bf16)
       ACT: p1 pair -> SBUF bf16 (1024-col copy)
       DVE: p12 = p1c * p2(PSUM) -> bf16 [128, 1024]
       PE:  selout[0:64]  += sel^T @ p12[chunk A]   (col-tile (0,0))
            selout[64:128]+= sel^T @ p12[chunk B]   (col-tile (0,64), concurrent)
  3. ACT: selout [128, NT] -> SBUF bf16, DMA out as [b, 2c, n].
"""

import numpy as np

# ---- problem constants (hardcoded; kernel.py must be self-contained) ----
B, T = 32, 8192
W, LAT, C = 128, 32, 64
STRIDE = 8
NWIN = 993            # output windows
NPAD = 1040           # winT free size (2*512 + 16)
N_CORES = 8
BPC = B // N_CORES    # batches per core
NTILES = 2            # 512-window tiles (512 + 481)
NJP = 8               # chunk pairs (16 chunks of 128 weight cols)

_CACHE = {}


def _build():
    """Build the per-core Bass program."""
    import concourse.bass as bass
    import concourse.mybir as mybir
    import concourse.tile as tile
    from concourse import bacc

    f32 = mybir.dt.float32
    bf16 = mybir.dt.bfloat16

    nc = bacc.Bacc(
        "TRN2",
        target_bir_lowering=False,
        debug=False,
        enable_asserts=False,
    )

    x_dram = nc.dram_tensor("x", [BPC, T + 256], bf16, kind="ExternalInput")
    w_dram = nc.dram_tensor("w", [W, 2 * LAT * C], bf16, kind="ExternalInput")
    sel_dram = nc.dram_tensor("sel", [W, C], bf16, kind="ExternalInput")
    out_dram = nc.dram_tensor("out", [BPC, 2 * C, NWIN], bf16, kind="ExternalOutput")

    with tile.TileContext(nc) as tc:
        with (
            tc.tile_pool(name="consts", bufs=1) as consts,
            tc.tile_pool(name="wins", bufs=2) as wins,
            tc.tile_pool(name="prods", bufs=4) as prods,
            tc.tile_pool(name="outs", bufs=2) as outs,
            tc.tile_pool(name="psum", bufs=1, space="PSUM") as psum,
        ):
            # Tile serializes every xbar-mode transition (transpose vs copy
            # DMA), so order matters: first the weight quarter + sel the
            # first matmuls need, then ALL im2col transposes back-to-back,
            # then the remaining const loads.
            w_sb = consts.tile([W, 2 * LAT * C], bf16)
            # ALL normal-mode loads first (weights + selector), THEN the
            # transposes: a normal<->xbar mode switch costs ~2us, so pay it
            # only once.  The whole normal group finishes in ~1us.
            nc.sync.dma_start(w_sb[:, 0:1024], w_dram.ap()[:, 0:1024])
            nc.sync.dma_start(w_sb[:, 2048:3072], w_dram.ap()[:, 2048:3072])
            sel_sb = consts.tile([W, C], bf16)
            nc.sync.dma_start(sel_sb[:], sel_dram.ap())
            for wq in (1, 3):
                nc.sync.dma_start(
                    w_sb[:, wq * 1024 : wq * 1024 + 1024],
                    w_dram.ap()[:, wq * 1024 : wq * 1024 + 1024],
                )
            winTs = []
            for b in range(BPC):
                wt = wins.tile([128, NPAD], bf16, name=f"winT{b}", tag="winT", bufs=4)
                v_main = bass.AP(
                    tensor=x_dram,
                    offset=b * (T + 256),
                    ap=[[STRIDE, NPAD], [1, W]],
                )
                nc.sync.dma_start_transpose(wt[:], v_main)
                winTs.append(wt)

            # dependency-free warmup matmuls: run while the startup DMA
            # chain is in flight so the PE clock gate (HAM) is already at
            # full rate when the real matmuls start
            warm_sb = consts.tile([128, 64], bf16)
            nc.gpsimd.memset(warm_sb[:], 0.0)
            warm_ps = psum.tile([128, 512], f32, tag="selout")
            for i in range(8):
                nc.tensor.matmul(
                    warm_ps[0:64, :],
                    warm_sb[:],
                    warm_sb[:, None, :].to_broadcast((128, 8, 64)),
                    start=(i == 0),
                    stop=(i == 7),
                )

            for b in range(BPC):
                winT = winTs[b]

                for t in range(NTILES):
                    # t=1 has only 481 real windows; don't compute the pad
                    NT = 512 if t == 0 else NWIN - 512
                    selout = psum.tile([128, 512], f32, tag="selout", bufs=1)
                    rhs1 = winT[:, t * 512 : t * 512 + NT]
                    rhs2 = winT[:, t * 512 + 16 : t * 512 + 16 + NT]
                    for jp in range(NJP):
                        # PSUM pool tiles: p1 single chunks (3 banks), p2
                        # pairs (2x2 banks); selout takes the 8th bank.
                        p2pair = psum.tile([128, 1024], f32, tag="p2pair", bufs=2)
                        p1c = prods.tile([128, 1024], bf16, tag="p1c", bufs=3)
                        p1s = []
                        for qi, j in enumerate((2 * jp, 2 * jp + 1)):
                            p1 = psum.tile([128, 512], f32, tag="p1", bufs=3)
                            nc.tensor.matmul(
                                p1[:, 0:NT],
                                w_sb[:, j * 128 : j * 128 + 128],
                                rhs1,
                                start=True,
                                stop=True,
                            )
                            nc.tensor.matmul(
                                p2pair[:, qi * 512 : qi * 512 + NT],
                                w_sb[:, 2048 + j * 128 : 2048 + j * 128 + 128],
                                rhs2,
                                start=True,
                                stop=True,
                            )
                            p1s.append(p1)
                        # ACT: exit p1 chunks to SBUF bf16 (halves of one tile)
                        for qi in range(2):
                            nc.scalar.copy(
                                p1c[:, qi * 512 : qi * 512 + NT], p1s[qi][:, 0:NT]
                            )
                        # DVE: p12 = p1c * p2 -> bf16 (one flat 1024-col op;
                        # for NT=481 the pad columns compute garbage that the
                        # selector never reads)
                        p12 = prods.tile([128, 1024], bf16, tag="p12", bufs=6)
                        nc.vector.tensor_mul(p12[:], p1c[:], p2pair[:])
                        # PE: twin col-tiled selector matmuls — chunk A into
                        # partitions 0-63, chunk B into 64-127, CONCURRENT
                        # in the array via col_grp masks.
                        nc.tensor.matmul(
                            selout[0:64, 0:NT],
                            sel_sb[:],
                            p12[:, 0:NT],
                            start=(jp == 0),
                            stop=(jp == NJP - 1),
                            tile_position=(0, 0),
                            skip_group_check=True,
                        )
                        nc.tensor.matmul(
                            selout[64:128, 0:NT],
                            sel_sb[:],
                            p12[:, 512 : 512 + NT],
                            start=(jp == 0),
                            stop=(jp == NJP - 1),
                            tile_position=(0, 64),
                            skip_group_check=True,
                        )
                    # exit both selector halves in one DVE op (ACT is the
                    # tighter engine in the steady-state loop)
                    st = outs.tile([128, 512], bf16, tag="st")
                    nc.vector.tensor_copy(st[:, 0:NT], selout[:, 0:NT])
                    n0 = t * 512
                    nc.sync.dma_start(
                        out_dram.ap()[b, :, n0 : n0 + NT], st[:, 0:NT]
                    )

    nc.compile()
    return nc


def _prep_inputs(X, weight1, weight2, bias):
    import ml_dtypes

    X = np.asarray(X, dtype=np.float32)
    weight1 = np.asarray(weight1, dtype=np.float32)
    weight2 = np.asarray(weight2, dtype=np.float32)

    # center over LAT, fold 1/LAT into proj1's weights; l-major layout
    w1c = weight1 - weight1.mean(axis=1, keepdims=True)
    w2c = weight2 - weight2.mean(axis=1, keepdims=True)
    w1p = (w1c / LAT).reshape(W, LAT * C)
    w2p = w2c.reshape(W, LAT * C)
    wcat = np.concatenate([w1p, w2p], axis=1).astype(ml_dtypes.bfloat16)

    xpad = np.zeros((B, T + 256), dtype=np.float32)
    xpad[:, :T] = X
    xb = xpad.astype(ml_dtypes.bfloat16)
    sel = (np.arange(W)[:, None] % C == np.arange(C)[None, :]).astype(
        ml_dtypes.bfloat16
    )

    in_maps = []
    for i in range(N_CORES):
        in_maps.append(
            {
                "x": np.ascontiguousarray(xb[i * BPC : (i + 1) * BPC]),
                "w": wcat,
                "sel": sel,
            }
        )
    return in_maps


def _merge_out(raw, bias):
    """[b, 2c, n] bf16 partial sums -> [b, n, c] fp32 (+bias)."""
    r = np.asarray(raw, dtype=np.float32)
    out = (r[:, :C, :] + r[:, C:, :]).transpose(0, 2, 1)
    return out + np.asarray(bias, dtype=np.float32)[None, None, :]


def run_with_results(X, weight1, weight2, bias, trace=False, trace_cores=None):
    from concourse import bass_utils

    if "nc" not in _CACHE:
        _CACHE["nc"] = _build()
    nc = _CACHE["nc"]
    in_maps = _prep_inputs(X, weight1, weight2, bias)
    res = bass_utils.run_bass_kernel_spmd(
        nc,
        in_maps,
        core_ids=list(range(N_CORES)),
        trace=trace,
        trace_cores=trace_cores,
    )
    out = np.concatenate(
        [_merge_out(res.results[i]["out"], bias) for i in range(N_CORES)], axis=0
    )
    return np.ascontiguousarray(out, dtype=np.float32), res


def kernel(**inputs):
    out, _ = run_with_results(
        inputs["X"], inputs["weight1"], inputs["weight2"], inputs["bias"]
    )
    return out
